# revision 4
# baseline (speedup 1.0000x reference)
"""Multi-head attention (B=16, S=1024, D=768, H=12) on 8 TRN2 NeuronCores.

Strategy: pure data parallelism — batch 16 is split 2-per-core; weights are
replicated. Each core runs an identical Bass/Tile program on its own x shard.

Optimizations vs the 438us v1 baseline (now ~331us):
  - all matmul inputs bf16 (x, W_qkv, q/k tiles): enables FWL weight loads,
    halves input DMA. PE accumulates fp32 so scores/out stay accurate.
  - every unit gets "fill" matmul work (qkgen of the NEXT unit, vgen of the
    next batch, proj of the previous batch), paced by an adaptive per-step
    budget and emitted BETWEEN the scores pair and the exp-dependent PV
    matmuls, so the in-order PE never stalls at the queue head waiting on
    ACT. This keeps PE duty high so the HAM clock gate stays at 2.4 GHz.
  - normalize split per-qc: qc0 normalization of a unit runs inside the same
    unit (step 9+), qc1 drips into the next unit. The last unit overlaps the
    final batch's qc0 projection; tt4-7 accumulate d-tiles 0..4 early and
    only the j=5 slice + bias-add run in the tail (proj_finish_tt).
  - reciprocal -> reciprocal_approx_fast (~5x faster, 18-bit accurate;
    NaNs if its APs start above partition 0 - keep denom tiles base-0).
  - bias via DVE tensor_add of a pre-broadcast [128,D] bias tile (no K=1
    bias matmuls), weight DMAs merged and spread across the scalar/gpsimd
    queues so the x shard + first head-pair's q/k land first.

Things measured to HURT (~1.2x global matmul-duration inflation, likely the
chip's P0 power throttle): issuing x-tile DMAs from the gpsimd or scalar
queues instead of sync, and padding the PV stationary operand to 128 weight
columns. Keep x loads on nc.sync and PV at M=65.

Per-core program (b in 0..1, head-pairs hp in 0..5):
  - v  = x @ W_v^T           [t, e] head-interleaved + ones col -> PV lhsT
  - qT2/kT2 [128, S]         two heads stacked on partitions (d-major)
  - scoresT[k,q] = k q^T     row-packed per head via tile_position (K=64)
  - exp on ACT (scale=1/8) -> bf16 SBUF tile
  - PV: out[dh+1, q] += v_ext.T @ exp   (row 64 accumulates the denom)
  - normalize: denom rows staged at partitions 0/32/64/96, approx-reciprocal
    per qc half, gpsimd partition_broadcast, DVE mult -> attn_outT (bf16)
  - y = attn_outT.T @ W_out^T + b_out  (bias added on DVE)
"""
import ml_dtypes
import numpy as np
import concourse.bacc as bacc
import concourse.tile as tile
from concourse import mybir
from concourse.bass_utils import run_bass_kernel_spmd

FP32 = mybir.dt.float32
BF16 = mybir.dt.bfloat16
EXP = mybir.ActivationFunctionType.Exp

B, S, D, H = 2, 1024, 768, 12       # per-core batch of 2
HP = H // 2                          # head pairs (6)
DT = D // 128                        # d tiles (6)
KT = S // 128                        # k tiles (8)
QC = S // 512                        # q chunks (2)
TT = S // 128                        # t tiles per batch (8)
N_CORES = 8

_CACHE = {}


class Fill:
    __slots__ = ("fn", "cost")

    def __init__(self, fn, cost):
        self.fn = fn
        self.cost = cost


def build_nc():
    nc = bacc.Bacc(trn_type="TRN2")
    # x pre-tiled host-side: row (b*DT+j)*128 + p = x[b, :, 128j+p],
    # so each [128, S] tile DMA is one fully contiguous 256KB block
    xT = nc.dram_tensor("xT", [B * D, S], BF16, kind="ExternalInput")
    wqkvT = nc.dram_tensor("wqkvT", [D, 3 * D], BF16, kind="ExternalInput")
    woutT = nc.dram_tensor("woutT", [D, D], BF16, kind="ExternalInput")
    biasf = nc.dram_tensor("biasf", [128, D], FP32, kind="ExternalInput")
    y = nc.dram_tensor("y", [B * S, D], FP32, kind="ExternalOutput")

    with tile.TileContext(nc) as tc:
        with (
            tc.tile_pool(name="wq", bufs=1) as p_wq,
            tc.tile_pool(name="wo", bufs=1) as p_wo,
            tc.tile_pool(name="cst", bufs=1) as p_cst,
            tc.tile_pool(name="xt", bufs=2) as p_xt,
            tc.tile_pool(name="vv", bufs=2) as p_v,
            tc.tile_pool(name="ao", bufs=2) as p_ao,
            tc.tile_pool(name="qk", bufs=4) as p_qk,
            tc.tile_pool(name="exp", bufs=3) as p_exp,
            tc.tile_pool(name="oc", bufs=6) as p_oc,
            tc.tile_pool(name="dn", bufs=2) as p_dn,
            tc.tile_pool(name="yy", bufs=8) as p_y,
            tc.tile_pool(name="rb", bufs=2) as p_rb,
            tc.tile_pool(name="r0", bufs=2) as p_r0,
            tc.tile_pool(name="sc", bufs=2, space="PSUM") as p_sc,
            tc.tile_pool(name="gen", bufs=2, space="PSUM") as p_gen,
            tc.tile_pool(name="oacc", bufs=2, space="PSUM") as p_oacc,
        ):
            wq = p_wq.tile([128, DT, 3 * D], BF16)
            wo = p_wo.tile([128, DT, D], BF16)
            bias_t = p_cst.tile([128, D], FP32)
            wqr = wqkvT.rearrange("(j p) e -> p j e", p=128)
            wor = woutT.rearrange("(j p) e -> p j e", p=128)
            # parallel queues: hp0 q/k + rest on scalar, v-cols on vector,
            # W_out + bias on gpsimd, x on sync (in load_xt below)
            nc.scalar.dma_start(wq[:, :, 0:128], wqr[:, :, 0:128])
            nc.scalar.dma_start(wq[:, :, D:D + 128], wqr[:, :, D:D + 128])
            nc.scalar.dma_start(wq[:, :, 2 * D:3 * D], wqr[:, :, 2 * D:3 * D])
            nc.scalar.dma_start(wq[:, :, 128:D], wqr[:, :, 128:D])
            nc.scalar.dma_start(
                wq[:, :, D + 128:2 * D], wqr[:, :, D + 128:2 * D])
            nc.scalar.dma_start(wo[:, :, :], wor[:, :, :])
            nc.gpsimd.dma_start(bias_t[:], biasf[:])

            xts, vs, aos = {}, {}, {}

            def load_xt(b):
                xt = p_xt.tile([128, DT, S], BF16, tag="xt")
                for j in range(DT):
                    r0 = (b * DT + j) * 128
                    nc.sync.dma_start(xt[:, j, :], xT[r0:r0 + 128, :])
                xts[b] = xt

            def alloc_v(b):
                v = p_v.tile([128, KT, H, 65], BF16, tag="vv")
                nc.vector.memset(v[:, :, :, 64], 1.0)
                vs[b] = v

            def vgen_fills(b):
                """16 fills: one [128,512-or-256] psum group + copy each."""
                fills = []
                for tt in range(TT):
                    for h0, nh in ((0, 8), (8, 4)):
                        def f(tt=tt, h0=h0, nh=nh, b=b):
                            xt, v = xts[b], vs[b]
                            vp = p_gen.tile([128, 512], FP32, tag="gen")
                            cw = nh * 64
                            for j in range(DT):
                                nc.tensor.matmul(
                                    vp[:, 0:cw],
                                    xt[:, j, tt * 128:(tt + 1) * 128],
                                    wq[:, j,
                                       2 * D + h0 * 64:2 * D + h0 * 64 + cw],
                                    start=(j == 0), stop=(j == DT - 1),
                                )
                            nc.vector.tensor_copy(
                                v[:, tt, h0:h0 + nh, 0:64],
                                vp[:, 0:cw].rearrange("p (h c) -> p h c", h=nh),
                            )
                        fills.append(Fill(f, 6))
                return fills

            proj_parts = {}

            def proj_fills_tt(b, tt, defer_j5=False):
                """2 fills for one token tile: y(b, tt) projection. With
                defer_j5, only d-tiles 0..4 accumulate (the j=5 slice of ao
                isn't normalized yet); proj_finish_tt adds the rest."""
                fills = []
                box = {}
                nj = DT - 1 if defer_j5 else DT
                for ci, (c0, cw) in enumerate(((0, 512), (512, 256))):
                    def f(tt=tt, ci=ci, c0=c0, cw=cw, b=b, box=box, nj=nj,
                          defer_j5=defer_j5):
                        ao = aos[b]
                        if ci == 0:
                            ys = p_y.tile([128, D], FP32, tag="yy")
                            box["ys"] = ys
                            proj_parts[(b, tt)] = ys
                        ys = box["ys"]
                        yp = p_gen.tile([128, 512], FP32, tag="gen")
                        for j in range(nj):
                            nc.tensor.matmul(
                                yp[:, 0:cw],
                                ao[:, j, tt * 128:(tt + 1) * 128],
                                wo[:, j, c0:c0 + cw],
                                start=(j == 0), stop=(j == nj - 1),
                            )
                        nc.vector.tensor_add(
                            ys[:, c0:c0 + cw], yp[:, 0:cw],
                            bias_t[:, c0:c0 + cw],
                        )
                        if ci == 1 and not defer_j5:
                            nc.sync.dma_start(
                                y[b * S + tt * 128:b * S + (tt + 1) * 128, :],
                                ys[:],
                            )
                    fills.append(Fill(f, nj))
                return fills

            def proj_finish_tt(b, tt):
                """Add the deferred j=5 contribution into the partial ys."""
                ao, ys = aos[b], proj_parts[(b, tt)]
                for c0, cw in ((0, 512), (512, 256)):
                    yp = p_gen.tile([128, 512], FP32, tag="gen")
                    nc.tensor.matmul(
                        yp[:, 0:cw],
                        ao[:, DT - 1, tt * 128:(tt + 1) * 128],
                        wo[:, DT - 1, c0:c0 + cw],
                        start=True, stop=True,
                    )
                    nc.vector.tensor_add(
                        ys[:, c0:c0 + cw], yp[:, 0:cw], ys[:, c0:c0 + cw])
                nc.sync.dma_start(
                    y[b * S + tt * 128:b * S + (tt + 1) * 128, :], ys[:])

            qkts = {}

            def qkgen_fills(b, hp):
                """4 fills building unit (b,hp)'s qT2/kT2 [128, S] tiles."""
                sqs = [
                    p_qk.tile([128, S], BF16, tag="qk", name=f"qk{b}_{hp}_{i}")
                    for i in range(2)
                ]
                qkts[(b, hp)] = sqs
                fills = []
                for part in range(2):  # 0 = q, 1 = k
                    for qc in range(QC):
                        def f(part=part, qc=qc, b=b, hp=hp):
                            qp = p_gen.tile([128, 512], FP32, tag="gen")
                            for j in range(DT):
                                nc.tensor.matmul(
                                    qp[:, :],
                                    wq[:, j,
                                       part * D + 128 * hp:part * D + 128 * (hp + 1)],
                                    xts[b][:, j, qc * 512:(qc + 1) * 512],
                                    start=(j == 0), stop=(j == DT - 1),
                                )
                            nc.vector.tensor_copy(
                                sqs[part][:, qc * 512:(qc + 1) * 512], qp[:, :]
                            )
                        fills.append(Fill(f, 6))
                return fills

            def unit(b, hp, fills, earlies, min_rate=0.0, late_fills=()):
                """One (batch, head-pair) attention unit: 16 scores/exp/PV
                steps with fill work dripped between the scores pair and the
                exp-dependent PV so the in-order PE never idles at the queue
                head. Returns the qc1-normalize closures (earlies for the
                next unit)."""
                v, ao = vs[b], aos[b]
                qT2, kT2 = qkts[(b, hp)]

                ocs, oaccs = {}, {}
                dns, dnrs = {}, {}

                def scores_exp(qc, kt):
                    sc = p_sc.tile([128, 1024], FP32, tag="sc")
                    nc.tensor.matmul(
                        sc[:, 0:512],
                        kT2[0:64, kt * 128:(kt + 1) * 128],
                        qT2[0:64, qc * 512:(qc + 1) * 512],
                        start=True, stop=True, tile_position=(0, 0),
                    )
                    nc.tensor.matmul(
                        sc[:, 512:1024],
                        kT2[64:128, kt * 128:(kt + 1) * 128],
                        qT2[64:128, qc * 512:(qc + 1) * 512],
                        start=True, stop=True, tile_position=(64, 0),
                    )
                    ex = p_exp.tile([128, 1024], BF16, tag="exp")
                    nc.scalar.activation(ex[:], sc[:], EXP, scale=0.125)
                    return ex

                def pv(qc, kt, ex):
                    if kt == 0:
                        o_a = p_oacc.tile([65, 512], FP32, tag="oacc")
                        o_b = p_oacc.tile([65, 512], FP32, tag="oacc")
                        oaccs[(qc, 0)] = o_a
                        oaccs[(qc, 1)] = o_b
                    nc.tensor.matmul(
                        oaccs[(qc, 0)][:], v[:, kt, 2 * hp, :], ex[:, 0:512],
                        start=(kt == 0), stop=(kt == KT - 1),
                    )
                    nc.tensor.matmul(
                        oaccs[(qc, 1)][:], v[:, kt, 2 * hp + 1, :],
                        ex[:, 512:1024],
                        start=(kt == 0), stop=(kt == KT - 1),
                    )
                    if kt == KT - 1:
                        # denom rows staged into a base-partition-0 tile
                        # (reciprocal_approx_fast NaNs on base-partition>0)
                        dn = p_dn.tile([64, 512], FP32, tag="dn")
                        nc.vector.memset(dn[:], 1.0)
                        dns[qc] = dn
                        for head in range(2):
                            oc = p_oc.tile([65, 512], FP32, tag="oc")
                            nc.vector.tensor_copy(oc[:], oaccs[(qc, head)][:])
                            nc.vector.tensor_copy(
                                dn[32 * head:32 * head + 1, :], oc[64:65, :]
                            )
                            ocs[(qc, head)] = oc

                rbs = {}

                def norm_closures(qc):
                    """5 closures: recip, then per head a (copy+broadcast)
                    stage and a mult stage (split so the DVE isn't parked
                    behind the 1.2us gpsimd broadcast)."""
                    cls = []
                    def recip(qc=qc):
                        dnr = p_dn.tile([64, 512], FP32, tag="dnr")
                        dnrs[qc] = dnr
                        nc.vector.reciprocal_approx_fast(
                            out=dnr[:], in_=dns[qc][:],
                        )
                    cls.append(recip)
                    for head in range(2):
                        def f1(qc=qc, head=head):
                            if head == 0:
                                src_row = dnrs[qc][0:1, :]
                            else:
                                r0 = p_r0.tile([1, 512], FP32, tag="r0")
                                nc.vector.tensor_copy(
                                    r0[:], dnrs[qc][32:33, :])
                                src_row = r0[:]
                            rb = p_rb.tile([64, 512], FP32, tag="rb")
                            nc.gpsimd.partition_broadcast(rb[:], src_row)
                            rbs[(qc, head)] = rb
                        def f2(qc=qc, head=head):
                            nc.vector.tensor_mul(
                                ao[64 * head:64 * (head + 1), hp,
                                   qc * 512:(qc + 1) * 512],
                                ocs[(qc, head)][0:64, :], rbs[(qc, head)],
                            )
                        cls.append((f1, f2))
                    (a1, a2), (b1, b2) = cls[1], cls[2]
                    return [cls[0], a1, b1, a2, b2]

                eq = list(earlies)
                fq = list(fills)
                debt = 0.0
                prev = None
                for step in range(QC * KT):
                    qc, kt = divmod(step, KT)
                    ex = scores_exp(qc, kt)
                    if step == 9:
                        # qc0 of this unit is fully PV'd (step 8): normalize
                        # it in-unit, then any late fills (kept FIFO-after
                        # the norm closures they depend on)
                        fq.extend(Fill(c, 0.5) for c in norm_closures(0))
                        fq.extend(late_fills)
                    if eq:
                        eq.pop(0)()
                    steps_left = QC * KT - step
                    total_left = sum(fl.cost for fl in fq)
                    debt = min(debt + max(total_left / steps_left, min_rate),
                               14.0)
                    while fq and fq[0].cost <= debt:
                        fl = fq.pop(0)
                        debt -= fl.cost
                        fl.fn()
                    if prev is not None:
                        pv(*prev)
                    prev = (qc, kt, ex)
                pv(*prev)
                for f in eq:
                    f()
                for fl in fq:
                    fl.fn()
                return norm_closures(1)

            # ---- schedule ----
            load_xt(0)
            alloc_v(0)
            qk00 = qkgen_fills(0, 0)
            for fl in qk00:
                fl.fn()

            # fills per unit: qkgen of the next unit everywhere, plus
            # vgen(next batch) on (b,2..5) and proj(prev batch) on (b,0..3).
            earlies = []
            norm_last = None
            for b in range(B):
                ao = p_ao.tile([128, DT, S], BF16, tag="ao")
                aos[b] = ao
                for hp in range(HP):
                    fills = []
                    late = []
                    min_rate = 0.0
                    if hp == 1 and b + 1 < B:
                        load_xt(b + 1)
                        alloc_v(b + 1)
                    if b == 0 and hp == 0:
                        fills += vgen_fills(0)
                        min_rate = 13.0
                    if not (b == B - 1 and hp == HP - 1):
                        nb, nhp = (b, hp + 1) if hp + 1 < HP else (b + 1, 0)
                        fills += qkgen_fills(nb, nhp)
                    if b + 1 < B and hp >= 1:
                        vg = vgen_fills(b + 1) if hp == 1 else vgs
                        vgs = vg
                        lo, hi = [(0, 3), (3, 6), (6, 9), (9, 12),
                                  (12, 16)][hp - 1]
                        fills += vg[lo:hi]
                    if b > 0 and hp <= 4:
                        tts = [(0, 1), (2, 3), (4, 5), (6,), (7,)][hp]
                        for t in tts:
                            fills += proj_fills_tt(b - 1, t)
                    if b == B - 1 and hp == HP - 1:
                        for tt in range(4):
                            late += proj_fills_tt(b, tt)
                        for tt in range(4, TT):
                            late += proj_fills_tt(b, tt, defer_j5=True)
                    norm_last = unit(b, hp, fills, earlies,
                                     min_rate=min_rate, late_fills=late)
                    earlies = norm_last
            for f in earlies:
                f()
            for tt in range(4, TT):
                proj_finish_tt(B - 1, tt)
    nc.finalize()
    return nc


def _marshal(x, W_qkv, W_out, b_out):
    bf = ml_dtypes.bfloat16
    wqkvT = np.ascontiguousarray(W_qkv.T).astype(bf)
    woutT = np.ascontiguousarray(W_out.T).astype(bf)
    bfull = np.ascontiguousarray(np.broadcast_to(
        np.asarray(b_out, np.float32).reshape(1, D), (128, D)))
    in_maps = []
    for c in range(N_CORES):
        xc = np.asarray(x)[B * c:B * (c + 1)].reshape(B * S, D).T
        xc = np.ascontiguousarray(
            xc.reshape(DT, 128, B, S).transpose(2, 0, 1, 3).reshape(B * D, S)
        ).astype(bf)
        in_maps.append({
            "xT": xc, "wqkvT": wqkvT, "woutT": woutT, "biasf": bfull,
        })
    return in_maps


def run(x, W_qkv, W_out, b_out, trace=False, **spmd_kwargs):
    if "nc" not in _CACHE:
        _CACHE["nc"] = build_nc()
    nc = _CACHE["nc"]
    in_maps = _marshal(x, W_qkv, W_out, b_out)
    res = run_bass_kernel_spmd(
        nc, in_maps, core_ids=list(range(N_CORES)), trace=trace, **spmd_kwargs
    )
    out = np.stack([res.results[c]["y"] for c in range(N_CORES)], axis=0)
    out = out.reshape(N_CORES * B, S, D)
    return out, res


def kernel(x, W_qkv, W_out, b_out):
    out, _ = run(x, W_qkv, W_out, b_out)
    return out


# revision 5
# speedup vs baseline: 1.0043x; 1.0043x over previous
"""Multi-head attention (B=16, S=1024, D=768, H=12) on 8 TRN2 NeuronCores.

Strategy: pure data parallelism — batch 16 is split 2-per-core; weights are
replicated. Each core runs an identical Bass/Tile program on its own x shard.

Optimizations vs the 438us v1 baseline (now ~331us):
  - all matmul inputs bf16 (x, W_qkv, q/k tiles): enables FWL weight loads,
    halves input DMA. PE accumulates fp32 so scores/out stay accurate.
  - every unit gets "fill" matmul work (qkgen of the NEXT unit, vgen of the
    next batch, proj of the previous batch), paced by an adaptive per-step
    budget and emitted BETWEEN the scores pair and the exp-dependent PV
    matmuls, so the in-order PE never stalls at the queue head waiting on
    ACT. This keeps PE duty high so the HAM clock gate stays at 2.4 GHz.
  - normalize split per-qc: qc0 normalization of a unit runs inside the same
    unit (step 9+), qc1 drips into the next unit. The last unit overlaps the
    final batch's qc0 projection; tt4-7 accumulate d-tiles 0..4 early and
    only the j=5 slice + bias-add run in the tail (proj_finish_tt).
  - reciprocal -> reciprocal_approx_fast (~5x faster, 18-bit accurate;
    NaNs if its APs start above partition 0 - keep denom tiles base-0).
  - bias via DVE tensor_add of a pre-broadcast [128,D] bias tile (no K=1
    bias matmuls), weight DMAs merged and spread across the scalar/gpsimd
    queues so the x shard + first head-pair's q/k land first.

Things measured to HURT (~1.2x global matmul-duration inflation, likely the
chip's P0 power throttle): issuing x-tile DMAs from the gpsimd or scalar
queues instead of sync, and padding the PV stationary operand to 128 weight
columns. Keep x loads on nc.sync and PV at M=65.

Per-core program (b in 0..1, head-pairs hp in 0..5):
  - v  = x @ W_v^T           [t, e] head-interleaved + ones col -> PV lhsT
  - qT2/kT2 [128, S]         two heads stacked on partitions (d-major)
  - scoresT[k,q] = k q^T     row-packed per head via tile_position (K=64)
  - exp on ACT (scale=1/8) -> bf16 SBUF tile
  - PV: out[dh+1, q] += v_ext.T @ exp   (row 64 accumulates the denom)
  - normalize: denom rows staged at partitions 0/32/64/96, approx-reciprocal
    per qc half, gpsimd partition_broadcast, DVE mult -> attn_outT (bf16)
  - y = attn_outT.T @ W_out^T + b_out  (bias added on DVE)
"""
import ml_dtypes
import numpy as np
import concourse.bacc as bacc
import concourse.tile as tile
from concourse import mybir
from concourse.bass_utils import run_bass_kernel_spmd

FP32 = mybir.dt.float32
BF16 = mybir.dt.bfloat16
EXP = mybir.ActivationFunctionType.Exp

B, S, D, H = 2, 1024, 768, 12       # per-core batch of 2
HP = H // 2                          # head pairs (6)
DT = D // 128                        # d tiles (6)
KT = S // 128                        # k tiles (8)
QC = S // 512                        # q chunks (2)
TT = S // 128                        # t tiles per batch (8)
N_CORES = 8

_CACHE = {}


class Fill:
    __slots__ = ("fn", "cost")

    def __init__(self, fn, cost):
        self.fn = fn
        self.cost = cost


def build_nc():
    nc = bacc.Bacc(trn_type="TRN2")
    xT = nc.dram_tensor("xT", [D, B * S], BF16, kind="ExternalInput")
    wqkvT = nc.dram_tensor("wqkvT", [D, 3 * D], BF16, kind="ExternalInput")
    woutT = nc.dram_tensor("woutT", [D, D], BF16, kind="ExternalInput")
    biasf = nc.dram_tensor("biasf", [128, D], FP32, kind="ExternalInput")
    y = nc.dram_tensor("y", [B * S, D], FP32, kind="ExternalOutput")

    with tile.TileContext(nc) as tc:
        with (
            tc.tile_pool(name="wq", bufs=1) as p_wq,
            tc.tile_pool(name="wo", bufs=1) as p_wo,
            tc.tile_pool(name="cst", bufs=1) as p_cst,
            tc.tile_pool(name="xt", bufs=2) as p_xt,
            tc.tile_pool(name="vv", bufs=2) as p_v,
            tc.tile_pool(name="ao", bufs=2) as p_ao,
            tc.tile_pool(name="qk", bufs=4) as p_qk,
            tc.tile_pool(name="exp", bufs=3) as p_exp,
            tc.tile_pool(name="oc", bufs=6) as p_oc,
            tc.tile_pool(name="dn", bufs=2) as p_dn,
            tc.tile_pool(name="yy", bufs=8) as p_y,
            tc.tile_pool(name="rb", bufs=2) as p_rb,
            tc.tile_pool(name="r0", bufs=2) as p_r0,
            tc.tile_pool(name="sc", bufs=2, space="PSUM") as p_sc,
            tc.tile_pool(name="gen", bufs=2, space="PSUM") as p_gen,
            tc.tile_pool(name="oacc", bufs=2, space="PSUM") as p_oacc,
        ):
            wq = p_wq.tile([128, DT, 3 * D], BF16)
            wo = p_wo.tile([128, DT, D], BF16)
            bias_t = p_cst.tile([128, D], FP32)
            wqr = wqkvT.rearrange("(j p) e -> p j e", p=128)
            wor = woutT.rearrange("(j p) e -> p j e", p=128)
            # parallel queues: hp0 q/k + rest on scalar, v-cols on vector,
            # W_out + bias on gpsimd, x on sync (in load_xt below)
            nc.scalar.dma_start(wq[:, :, 0:128], wqr[:, :, 0:128])
            nc.scalar.dma_start(wq[:, :, D:D + 128], wqr[:, :, D:D + 128])
            nc.scalar.dma_start(wq[:, :, 2 * D:3 * D], wqr[:, :, 2 * D:3 * D])
            nc.scalar.dma_start(wq[:, :, 128:D], wqr[:, :, 128:D])
            nc.scalar.dma_start(
                wq[:, :, D + 128:2 * D], wqr[:, :, D + 128:2 * D])
            nc.scalar.dma_start(wo[:, :, :], wor[:, :, :])
            nc.gpsimd.dma_start(bias_t[:], biasf[:])

            xts, vs, aos = {}, {}, {}

            def load_xt(b):
                xt = p_xt.tile([128, DT, S], BF16, tag="xt")
                for j in range(DT):
                    nc.sync.dma_start(
                        xt[:, j, :], xT[128 * j:128 * (j + 1), b * S:(b + 1) * S]
                    )
                xts[b] = xt

            def alloc_v(b):
                v = p_v.tile([128, KT, H, 65], BF16, tag="vv")
                nc.vector.memset(v[:, :, :, 64], 1.0)
                vs[b] = v

            def vgen_fills(b):
                """16 fills: one [128,512-or-256] psum group + copy each."""
                fills = []
                for tt in range(TT):
                    for h0, nh in ((0, 8), (8, 4)):
                        def f(tt=tt, h0=h0, nh=nh, b=b):
                            xt, v = xts[b], vs[b]
                            vp = p_gen.tile([128, 512], FP32, tag="gen")
                            cw = nh * 64
                            for j in range(DT):
                                nc.tensor.matmul(
                                    vp[:, 0:cw],
                                    xt[:, j, tt * 128:(tt + 1) * 128],
                                    wq[:, j,
                                       2 * D + h0 * 64:2 * D + h0 * 64 + cw],
                                    start=(j == 0), stop=(j == DT - 1),
                                )
                            nc.vector.tensor_copy(
                                v[:, tt, h0:h0 + nh, 0:64],
                                vp[:, 0:cw].rearrange("p (h c) -> p h c", h=nh),
                            )
                        fills.append(Fill(f, 6))
                return fills

            proj_parts = {}

            def proj_fills_tt(b, tt, defer_j5=False):
                """2 fills for one token tile: y(b, tt) projection. With
                defer_j5, only d-tiles 0..4 accumulate (the j=5 slice of ao
                isn't normalized yet); proj_finish_tt adds the rest."""
                fills = []
                box = {}
                nj = DT - 1 if defer_j5 else DT
                for ci, (c0, cw) in enumerate(((0, 512), (512, 256))):
                    def f(tt=tt, ci=ci, c0=c0, cw=cw, b=b, box=box, nj=nj,
                          defer_j5=defer_j5):
                        ao = aos[b]
                        if ci == 0:
                            ys = p_y.tile([128, D], FP32, tag="yy")
                            box["ys"] = ys
                            proj_parts[(b, tt)] = ys
                        ys = box["ys"]
                        yp = p_gen.tile([128, 512], FP32, tag="gen")
                        for j in range(nj):
                            nc.tensor.matmul(
                                yp[:, 0:cw],
                                ao[:, j, tt * 128:(tt + 1) * 128],
                                wo[:, j, c0:c0 + cw],
                                start=(j == 0), stop=(j == nj - 1),
                            )
                        nc.vector.tensor_add(
                            ys[:, c0:c0 + cw], yp[:, 0:cw],
                            bias_t[:, c0:c0 + cw],
                        )
                        if ci == 1 and not defer_j5:
                            nc.sync.dma_start(
                                y[b * S + tt * 128:b * S + (tt + 1) * 128, :],
                                ys[:],
                            )
                    fills.append(Fill(f, nj))
                return fills

            def proj_finish_tt(b, tt):
                """Add the deferred j=5 contribution into the partial ys."""
                ao, ys = aos[b], proj_parts[(b, tt)]
                for c0, cw in ((0, 512), (512, 256)):
                    yp = p_gen.tile([128, 512], FP32, tag="gen")
                    nc.tensor.matmul(
                        yp[:, 0:cw],
                        ao[:, DT - 1, tt * 128:(tt + 1) * 128],
                        wo[:, DT - 1, c0:c0 + cw],
                        start=True, stop=True,
                    )
                    nc.vector.tensor_add(
                        ys[:, c0:c0 + cw], yp[:, 0:cw], ys[:, c0:c0 + cw])
                nc.sync.dma_start(
                    y[b * S + tt * 128:b * S + (tt + 1) * 128, :], ys[:])

            qkts = {}

            def qkgen_fills(b, hp):
                """4 fills building unit (b,hp)'s qT2/kT2 [128, S] tiles."""
                sqs = [
                    p_qk.tile([128, S], BF16, tag="qk", name=f"qk{b}_{hp}_{i}")
                    for i in range(2)
                ]
                qkts[(b, hp)] = sqs
                fills = []
                for part in range(2):  # 0 = q, 1 = k
                    for qc in range(QC):
                        def f(part=part, qc=qc, b=b, hp=hp):
                            qp = p_gen.tile([128, 512], FP32, tag="gen")
                            for j in range(DT):
                                nc.tensor.matmul(
                                    qp[:, :],
                                    wq[:, j,
                                       part * D + 128 * hp:part * D + 128 * (hp + 1)],
                                    xts[b][:, j, qc * 512:(qc + 1) * 512],
                                    start=(j == 0), stop=(j == DT - 1),
                                )
                            nc.vector.tensor_copy(
                                sqs[part][:, qc * 512:(qc + 1) * 512], qp[:, :]
                            )
                        fills.append(Fill(f, 6))
                return fills

            def unit(b, hp, fills, earlies, min_rate=0.0, late_fills=()):
                """One (batch, head-pair) attention unit: 16 scores/exp/PV
                steps with fill work dripped between the scores pair and the
                exp-dependent PV so the in-order PE never idles at the queue
                head. Returns the qc1-normalize closures (earlies for the
                next unit)."""
                v, ao = vs[b], aos[b]
                qT2, kT2 = qkts[(b, hp)]

                ocs, oaccs = {}, {}
                dns, dnrs = {}, {}

                def scores_exp(qc, kt):
                    sc = p_sc.tile([128, 1024], FP32, tag="sc")
                    nc.tensor.matmul(
                        sc[:, 0:512],
                        kT2[0:64, kt * 128:(kt + 1) * 128],
                        qT2[0:64, qc * 512:(qc + 1) * 512],
                        start=True, stop=True, tile_position=(0, 0),
                    )
                    nc.tensor.matmul(
                        sc[:, 512:1024],
                        kT2[64:128, kt * 128:(kt + 1) * 128],
                        qT2[64:128, qc * 512:(qc + 1) * 512],
                        start=True, stop=True, tile_position=(64, 0),
                    )
                    ex = p_exp.tile([128, 1024], BF16, tag="exp")
                    nc.scalar.activation(ex[:], sc[:], EXP, scale=0.125)
                    return ex

                def pv(qc, kt, ex):
                    if kt == 0:
                        o_a = p_oacc.tile([65, 512], FP32, tag="oacc")
                        o_b = p_oacc.tile([65, 512], FP32, tag="oacc")
                        oaccs[(qc, 0)] = o_a
                        oaccs[(qc, 1)] = o_b
                    nc.tensor.matmul(
                        oaccs[(qc, 0)][:], v[:, kt, 2 * hp, :], ex[:, 0:512],
                        start=(kt == 0), stop=(kt == KT - 1),
                    )
                    nc.tensor.matmul(
                        oaccs[(qc, 1)][:], v[:, kt, 2 * hp + 1, :],
                        ex[:, 512:1024],
                        start=(kt == 0), stop=(kt == KT - 1),
                    )
                    if kt == KT - 1:
                        # denom rows staged into a base-partition-0 tile
                        # (reciprocal_approx_fast NaNs on base-partition>0)
                        dn = p_dn.tile([64, 512], FP32, tag="dn")
                        nc.vector.memset(dn[:], 1.0)
                        dns[qc] = dn
                        for head in range(2):
                            oc = p_oc.tile([65, 512], FP32, tag="oc")
                            nc.vector.tensor_copy(oc[:], oaccs[(qc, head)][:])
                            nc.vector.tensor_copy(
                                dn[32 * head:32 * head + 1, :], oc[64:65, :]
                            )
                            ocs[(qc, head)] = oc

                rbs = {}

                def norm_closures(qc):
                    """5 closures: recip, then per head a (copy+broadcast)
                    stage and a mult stage (split so the DVE isn't parked
                    behind the 1.2us gpsimd broadcast)."""
                    cls = []
                    def recip(qc=qc):
                        dnr = p_dn.tile([64, 512], FP32, tag="dnr")
                        dnrs[qc] = dnr
                        nc.vector.reciprocal_approx_fast(
                            out=dnr[:], in_=dns[qc][:],
                        )
                    cls.append(recip)
                    for head in range(2):
                        def f1(qc=qc, head=head):
                            if head == 0:
                                src_row = dnrs[qc][0:1, :]
                            else:
                                r0 = p_r0.tile([1, 512], FP32, tag="r0")
                                nc.vector.tensor_copy(
                                    r0[:], dnrs[qc][32:33, :])
                                src_row = r0[:]
                            rb = p_rb.tile([64, 512], FP32, tag="rb")
                            nc.gpsimd.partition_broadcast(rb[:], src_row)
                            rbs[(qc, head)] = rb
                        def f2(qc=qc, head=head):
                            nc.vector.tensor_mul(
                                ao[64 * head:64 * (head + 1), hp,
                                   qc * 512:(qc + 1) * 512],
                                ocs[(qc, head)][0:64, :], rbs[(qc, head)],
                            )
                        cls.append((f1, f2))
                    (a1, a2), (b1, b2) = cls[1], cls[2]
                    return [cls[0], a1, b1, a2, b2]

                eq = list(earlies)
                fq = list(fills)
                debt = 0.0
                prev = None
                for step in range(QC * KT):
                    qc, kt = divmod(step, KT)
                    ex = scores_exp(qc, kt)
                    if step == 9:
                        # qc0 of this unit is fully PV'd (step 8): normalize
                        # it in-unit, then any late fills (kept FIFO-after
                        # the norm closures they depend on)
                        fq.extend(Fill(c, 0.5) for c in norm_closures(0))
                        fq.extend(late_fills)
                    if eq:
                        eq.pop(0)()
                    steps_left = QC * KT - step
                    total_left = sum(fl.cost for fl in fq)
                    debt = min(debt + max(total_left / steps_left, min_rate),
                               14.0)
                    while fq and fq[0].cost <= debt:
                        fl = fq.pop(0)
                        debt -= fl.cost
                        fl.fn()
                    if prev is not None:
                        pv(*prev)
                    prev = (qc, kt, ex)
                pv(*prev)
                for f in eq:
                    f()
                for fl in fq:
                    fl.fn()
                return norm_closures(1)

            # ---- schedule ----
            load_xt(0)
            alloc_v(0)
            qk00 = qkgen_fills(0, 0)
            for fl in qk00:
                fl.fn()

            # fills per unit: qkgen of the next unit everywhere, plus
            # vgen(next batch) on (b,2..5) and proj(prev batch) on (b,0..3).
            earlies = []
            norm_last = None
            for b in range(B):
                ao = p_ao.tile([128, DT, S], BF16, tag="ao")
                aos[b] = ao
                for hp in range(HP):
                    fills = []
                    late = []
                    min_rate = 0.0
                    if hp == 1 and b + 1 < B:
                        load_xt(b + 1)
                        alloc_v(b + 1)
                    if b == 0 and hp == 0:
                        fills += vgen_fills(0)
                        min_rate = 13.0
                    if not (b == B - 1 and hp == HP - 1):
                        nb, nhp = (b, hp + 1) if hp + 1 < HP else (b + 1, 0)
                        fills += qkgen_fills(nb, nhp)
                    if b + 1 < B and hp >= 1:
                        vg = vgen_fills(b + 1) if hp == 1 else vgs
                        vgs = vg
                        lo, hi = [(0, 3), (3, 6), (6, 9), (9, 12),
                                  (12, 16)][hp - 1]
                        fills += vg[lo:hi]
                    if b > 0 and hp <= 4:
                        tts = [(0, 1), (2, 3), (4, 5), (6,), (7,)][hp]
                        for t in tts:
                            fills += proj_fills_tt(b - 1, t)
                    if b == B - 1 and hp == HP - 1:
                        for tt in range(4):
                            late += proj_fills_tt(b, tt)
                        for tt in range(4, TT):
                            late += proj_fills_tt(b, tt, defer_j5=True)
                    norm_last = unit(b, hp, fills, earlies,
                                     min_rate=min_rate, late_fills=late)
                    earlies = norm_last
            for f in earlies:
                f()
            for tt in range(4, TT):
                proj_finish_tt(B - 1, tt)
    nc.finalize()
    return nc


def _marshal(x, W_qkv, W_out, b_out):
    bf = ml_dtypes.bfloat16
    wqkvT = np.ascontiguousarray(W_qkv.T).astype(bf)
    woutT = np.ascontiguousarray(W_out.T).astype(bf)
    bfull = np.ascontiguousarray(np.broadcast_to(
        np.asarray(b_out, np.float32).reshape(1, D), (128, D)))
    in_maps = []
    for c in range(N_CORES):
        xc = np.ascontiguousarray(
            np.asarray(x)[B * c:B * (c + 1)].reshape(B * S, D).T
        ).astype(bf)
        in_maps.append({
            "xT": xc, "wqkvT": wqkvT, "woutT": woutT, "biasf": bfull,
        })
    return in_maps


def run(x, W_qkv, W_out, b_out, trace=False, **spmd_kwargs):
    if "nc" not in _CACHE:
        _CACHE["nc"] = build_nc()
    nc = _CACHE["nc"]
    in_maps = _marshal(x, W_qkv, W_out, b_out)
    res = run_bass_kernel_spmd(
        nc, in_maps, core_ids=list(range(N_CORES)), trace=trace, **spmd_kwargs
    )
    out = np.stack([res.results[c]["y"] for c in range(N_CORES)], axis=0)
    out = out.reshape(N_CORES * B, S, D)
    return out, res


def kernel(x, W_qkv, W_out, b_out):
    out, _ = run(x, W_qkv, W_out, b_out)
    return out


# revision 6
# speedup vs baseline: 1.0081x; 1.0038x over previous
"""Multi-head attention (B=16, S=1024, D=768, H=12) on 8 TRN2 NeuronCores.

Strategy: pure data parallelism — batch 16 is split 2-per-core; weights are
replicated. Each core runs an identical Bass/Tile program on its own x shard.

Optimizations vs the 438us v1 baseline (now ~331us):
  - all matmul inputs bf16 (x, W_qkv, q/k tiles): enables FWL weight loads,
    halves input DMA. PE accumulates fp32 so scores/out stay accurate.
  - every unit gets "fill" matmul work (qkgen of the NEXT unit, vgen of the
    next batch, proj of the previous batch), paced by an adaptive per-step
    budget and emitted BETWEEN the scores pair and the exp-dependent PV
    matmuls, so the in-order PE never stalls at the queue head waiting on
    ACT. This keeps PE duty high so the HAM clock gate stays at 2.4 GHz.
  - normalize split per-qc: qc0 normalization of a unit runs inside the same
    unit (step 9+), qc1 drips into the next unit. The last unit overlaps the
    final batch's qc0 projection; tt4-7 accumulate d-tiles 0..4 early and
    only the j=5 slice + bias-add run in the tail (proj_finish_tt).
  - reciprocal -> reciprocal_approx_fast (~5x faster, 18-bit accurate;
    NaNs if its APs start above partition 0 - keep denom tiles base-0).
  - bias via DVE tensor_add of a pre-broadcast [128,D] bias tile (no K=1
    bias matmuls), weight DMAs merged and spread across the scalar/gpsimd
    queues so the x shard + first head-pair's q/k land first.

Measurement note: runs intermittently land in a ~1.19x chip-wide slow
mode (all engines inflate, including fixed-clock ACT) — re-run before
judging any variant. Mode-adjusted, padding the PV stationary operand to
128 weight columns was still ~+7us worse (extra LDWEIGHTS columns); fill
granularity, step pair-batching, weight-sharing, DMA queue fan-out, and
contiguous x marshaling all measured neutral — the ~53us of per-matmul
friction above the 1-col/cycle streaming floor is schedule-invariant.

Per-core program (b in 0..1, head-pairs hp in 0..5):
  - v  = x @ W_v^T           [t, e] head-interleaved + ones col -> PV lhsT
  - qT2/kT2 [128, S]         two heads stacked on partitions (d-major)
  - scoresT[k,q] = k q^T     row-packed per head via tile_position (K=64)
  - exp on ACT (scale=1/8) -> bf16 SBUF tile
  - PV: out[dh+1, q] += v_ext.T @ exp   (row 64 accumulates the denom)
  - normalize: denom rows staged at partitions 0/32/64/96, approx-reciprocal
    per qc half, gpsimd partition_broadcast, DVE mult -> attn_outT (bf16)
  - y = attn_outT.T @ W_out^T + b_out  (bias added on DVE)
"""
import ml_dtypes
import numpy as np
import concourse.bacc as bacc
import concourse.tile as tile
from concourse import mybir
from concourse.bass_utils import run_bass_kernel_spmd

FP32 = mybir.dt.float32
BF16 = mybir.dt.bfloat16
EXP = mybir.ActivationFunctionType.Exp

B, S, D, H = 2, 1024, 768, 12       # per-core batch of 2
HP = H // 2                          # head pairs (6)
DT = D // 128                        # d tiles (6)
KT = S // 128                        # k tiles (8)
QC = S // 512                        # q chunks (2)
TT = S // 128                        # t tiles per batch (8)
N_CORES = 8

_CACHE = {}


class Fill:
    __slots__ = ("fn", "cost")

    def __init__(self, fn, cost):
        self.fn = fn
        self.cost = cost


def build_nc():
    nc = bacc.Bacc(trn_type="TRN2")
    xT = nc.dram_tensor("xT", [D, B * S], BF16, kind="ExternalInput")
    wqkvT = nc.dram_tensor("wqkvT", [D, 3 * D], BF16, kind="ExternalInput")
    woutT = nc.dram_tensor("woutT", [D, D], BF16, kind="ExternalInput")
    biasf = nc.dram_tensor("biasf", [128, D], FP32, kind="ExternalInput")
    y = nc.dram_tensor("y", [B * S, D], FP32, kind="ExternalOutput")

    with tile.TileContext(nc) as tc:
        with (
            tc.tile_pool(name="wq", bufs=1) as p_wq,
            tc.tile_pool(name="wo", bufs=1) as p_wo,
            tc.tile_pool(name="cst", bufs=1) as p_cst,
            tc.tile_pool(name="xt", bufs=2) as p_xt,
            tc.tile_pool(name="vv", bufs=2) as p_v,
            tc.tile_pool(name="ao", bufs=2) as p_ao,
            tc.tile_pool(name="qk", bufs=4) as p_qk,
            tc.tile_pool(name="exp", bufs=3) as p_exp,
            tc.tile_pool(name="oc", bufs=6) as p_oc,
            tc.tile_pool(name="dn", bufs=2) as p_dn,
            tc.tile_pool(name="yy", bufs=8) as p_y,
            tc.tile_pool(name="rb", bufs=2) as p_rb,
            tc.tile_pool(name="r0", bufs=2) as p_r0,
            tc.tile_pool(name="sc", bufs=2, space="PSUM") as p_sc,
            tc.tile_pool(name="gen", bufs=2, space="PSUM") as p_gen,
            tc.tile_pool(name="oacc", bufs=2, space="PSUM") as p_oacc,
        ):
            wq = p_wq.tile([128, DT, 3 * D], BF16)
            wo = p_wo.tile([128, DT, D], BF16)
            bias_t = p_cst.tile([128, D], FP32)
            wqr = wqkvT.rearrange("(j p) e -> p j e", p=128)
            wor = woutT.rearrange("(j p) e -> p j e", p=128)
            # parallel queues: hp0 q/k + rest on scalar, v-cols on vector,
            # W_out + bias on gpsimd, x on sync (in load_xt below)
            nc.scalar.dma_start(wq[:, :, 0:128], wqr[:, :, 0:128])
            nc.scalar.dma_start(wq[:, :, D:D + 128], wqr[:, :, D:D + 128])
            nc.scalar.dma_start(wq[:, :, 2 * D:3 * D], wqr[:, :, 2 * D:3 * D])
            nc.scalar.dma_start(wq[:, :, 128:D], wqr[:, :, 128:D])
            nc.scalar.dma_start(
                wq[:, :, D + 128:2 * D], wqr[:, :, D + 128:2 * D])
            nc.scalar.dma_start(wo[:, :, :], wor[:, :, :])
            nc.gpsimd.dma_start(bias_t[:], biasf[:])

            xts, vs, aos = {}, {}, {}

            def load_xt(b):
                xt = p_xt.tile([128, DT, S], BF16, tag="xt")
                for j in range(DT):
                    nc.sync.dma_start(
                        xt[:, j, :], xT[128 * j:128 * (j + 1), b * S:(b + 1) * S]
                    )
                xts[b] = xt

            def alloc_v(b):
                v = p_v.tile([128, KT, H, 65], BF16, tag="vv")
                nc.vector.memset(v[:, :, :, 64], 1.0)
                vs[b] = v

            def vgen_fills(b):
                """16 fills: one [128,512-or-256] psum group + copy each."""
                fills = []
                for tt in range(TT):
                    for h0, nh in ((0, 8), (8, 4)):
                        def f(tt=tt, h0=h0, nh=nh, b=b):
                            xt, v = xts[b], vs[b]
                            vp = p_gen.tile([128, 512], FP32, tag="gen")
                            cw = nh * 64
                            for j in range(DT):
                                nc.tensor.matmul(
                                    vp[:, 0:cw],
                                    xt[:, j, tt * 128:(tt + 1) * 128],
                                    wq[:, j,
                                       2 * D + h0 * 64:2 * D + h0 * 64 + cw],
                                    start=(j == 0), stop=(j == DT - 1),
                                )
                            nc.vector.tensor_copy(
                                v[:, tt, h0:h0 + nh, 0:64],
                                vp[:, 0:cw].rearrange("p (h c) -> p h c", h=nh),
                            )
                        fills.append(Fill(f, 6))
                return fills

            proj_parts = {}

            def proj_fills_tt(b, tt, defer_j5=False):
                """2 fills for one token tile: y(b, tt) projection. With
                defer_j5, only d-tiles 0..4 accumulate (the j=5 slice of ao
                isn't normalized yet); proj_finish_tt adds the rest."""
                fills = []
                box = {}
                nj = DT - 1 if defer_j5 else DT
                for ci, (c0, cw) in enumerate(((0, 512), (512, 256))):
                    def f(tt=tt, ci=ci, c0=c0, cw=cw, b=b, box=box, nj=nj,
                          defer_j5=defer_j5):
                        ao = aos[b]
                        if ci == 0:
                            ys = p_y.tile([128, D], FP32, tag="yy")
                            box["ys"] = ys
                            proj_parts[(b, tt)] = ys
                        ys = box["ys"]
                        yp = p_gen.tile([128, 512], FP32, tag="gen")
                        for j in range(nj):
                            nc.tensor.matmul(
                                yp[:, 0:cw],
                                ao[:, j, tt * 128:(tt + 1) * 128],
                                wo[:, j, c0:c0 + cw],
                                start=(j == 0), stop=(j == nj - 1),
                            )
                        nc.vector.tensor_add(
                            ys[:, c0:c0 + cw], yp[:, 0:cw],
                            bias_t[:, c0:c0 + cw],
                        )
                        if ci == 1 and not defer_j5:
                            nc.sync.dma_start(
                                y[b * S + tt * 128:b * S + (tt + 1) * 128, :],
                                ys[:],
                            )
                    fills.append(Fill(f, nj))
                return fills

            def proj_finish_tt(b, tt):
                """Add the deferred j=5 contribution into the partial ys."""
                ao, ys = aos[b], proj_parts[(b, tt)]
                for c0, cw in ((0, 512), (512, 256)):
                    yp = p_gen.tile([128, 512], FP32, tag="gen")
                    nc.tensor.matmul(
                        yp[:, 0:cw],
                        ao[:, DT - 1, tt * 128:(tt + 1) * 128],
                        wo[:, DT - 1, c0:c0 + cw],
                        start=True, stop=True,
                    )
                    nc.vector.tensor_add(
                        ys[:, c0:c0 + cw], yp[:, 0:cw], ys[:, c0:c0 + cw])
                nc.sync.dma_start(
                    y[b * S + tt * 128:b * S + (tt + 1) * 128, :], ys[:])

            qkts = {}

            def qkgen_fills(b, hp):
                """4 fills building unit (b,hp)'s qT2/kT2 [128, S] tiles."""
                sqs = [
                    p_qk.tile([128, S], BF16, tag="qk", name=f"qk{b}_{hp}_{i}")
                    for i in range(2)
                ]
                qkts[(b, hp)] = sqs
                fills = []
                for part in range(2):  # 0 = q, 1 = k
                    for qc in range(QC):
                        def f(part=part, qc=qc, b=b, hp=hp):
                            qp = p_gen.tile([128, 512], FP32, tag="gen")
                            for j in range(DT):
                                nc.tensor.matmul(
                                    qp[:, :],
                                    wq[:, j,
                                       part * D + 128 * hp:part * D + 128 * (hp + 1)],
                                    xts[b][:, j, qc * 512:(qc + 1) * 512],
                                    start=(j == 0), stop=(j == DT - 1),
                                )
                            nc.vector.tensor_copy(
                                sqs[part][:, qc * 512:(qc + 1) * 512], qp[:, :]
                            )
                        fills.append(Fill(f, 6))
                return fills

            def unit(b, hp, fills, earlies, min_rate=0.0, late_fills=()):
                """One (batch, head-pair) attention unit: 16 scores/exp/PV
                steps with fill work dripped between the scores pair and the
                exp-dependent PV so the in-order PE never idles at the queue
                head. Returns the qc1-normalize closures (earlies for the
                next unit)."""
                v, ao = vs[b], aos[b]
                qT2, kT2 = qkts[(b, hp)]

                ocs, oaccs = {}, {}
                dns, dnrs = {}, {}

                def scores_exp(qc, kt):
                    sc = p_sc.tile([128, 1024], FP32, tag="sc")
                    nc.tensor.matmul(
                        sc[:, 0:512],
                        kT2[0:64, kt * 128:(kt + 1) * 128],
                        qT2[0:64, qc * 512:(qc + 1) * 512],
                        start=True, stop=True, tile_position=(0, 0),
                    )
                    nc.tensor.matmul(
                        sc[:, 512:1024],
                        kT2[64:128, kt * 128:(kt + 1) * 128],
                        qT2[64:128, qc * 512:(qc + 1) * 512],
                        start=True, stop=True, tile_position=(64, 0),
                    )
                    ex = p_exp.tile([128, 1024], BF16, tag="exp")
                    nc.scalar.activation(ex[:], sc[:], EXP, scale=0.125)
                    return ex

                def pv(qc, kt, ex):
                    if kt == 0:
                        o_a = p_oacc.tile([65, 512], FP32, tag="oacc")
                        o_b = p_oacc.tile([65, 512], FP32, tag="oacc")
                        oaccs[(qc, 0)] = o_a
                        oaccs[(qc, 1)] = o_b
                    nc.tensor.matmul(
                        oaccs[(qc, 0)][:], v[:, kt, 2 * hp, :], ex[:, 0:512],
                        start=(kt == 0), stop=(kt == KT - 1),
                    )
                    nc.tensor.matmul(
                        oaccs[(qc, 1)][:], v[:, kt, 2 * hp + 1, :],
                        ex[:, 512:1024],
                        start=(kt == 0), stop=(kt == KT - 1),
                    )
                    if kt == KT - 1:
                        # denom rows staged into a base-partition-0 tile
                        # (reciprocal_approx_fast NaNs on base-partition>0)
                        dn = p_dn.tile([64, 512], FP32, tag="dn")
                        nc.vector.memset(dn[:], 1.0)
                        dns[qc] = dn
                        for head in range(2):
                            oc = p_oc.tile([65, 512], FP32, tag="oc")
                            nc.vector.tensor_copy(oc[:], oaccs[(qc, head)][:])
                            nc.vector.tensor_copy(
                                dn[32 * head:32 * head + 1, :], oc[64:65, :]
                            )
                            ocs[(qc, head)] = oc

                rbs = {}

                def norm_closures(qc):
                    """5 closures: recip, then per head a (copy+broadcast)
                    stage and a mult stage (split so the DVE isn't parked
                    behind the 1.2us gpsimd broadcast)."""
                    cls = []
                    def recip(qc=qc):
                        dnr = p_dn.tile([64, 512], FP32, tag="dnr")
                        dnrs[qc] = dnr
                        nc.vector.reciprocal_approx_fast(
                            out=dnr[:], in_=dns[qc][:],
                        )
                    cls.append(recip)
                    for head in range(2):
                        def f1(qc=qc, head=head):
                            if head == 0:
                                src_row = dnrs[qc][0:1, :]
                            else:
                                r0 = p_r0.tile([1, 512], FP32, tag="r0")
                                nc.vector.tensor_copy(
                                    r0[:], dnrs[qc][32:33, :])
                                src_row = r0[:]
                            rb = p_rb.tile([64, 512], FP32, tag="rb")
                            nc.gpsimd.partition_broadcast(rb[:], src_row)
                            rbs[(qc, head)] = rb
                        def f2(qc=qc, head=head):
                            nc.vector.tensor_mul(
                                ao[64 * head:64 * (head + 1), hp,
                                   qc * 512:(qc + 1) * 512],
                                ocs[(qc, head)][0:64, :], rbs[(qc, head)],
                            )
                        cls.append((f1, f2))
                    (a1, a2), (b1, b2) = cls[1], cls[2]
                    return [cls[0], a1, b1, a2, b2]

                eq = list(earlies)
                fq = list(fills)
                debt = 0.0
                prev = None
                for step in range(QC * KT):
                    qc, kt = divmod(step, KT)
                    ex = scores_exp(qc, kt)
                    if step == 9:
                        # qc0 of this unit is fully PV'd (step 8): normalize
                        # it in-unit, then any late fills (kept FIFO-after
                        # the norm closures they depend on)
                        fq.extend(Fill(c, 0.5) for c in norm_closures(0))
                        fq.extend(late_fills)
                    if eq:
                        eq.pop(0)()
                    steps_left = QC * KT - step
                    total_left = sum(fl.cost for fl in fq)
                    debt = min(debt + max(total_left / steps_left, min_rate),
                               14.0)
                    while fq and fq[0].cost <= debt:
                        fl = fq.pop(0)
                        debt -= fl.cost
                        fl.fn()
                    if prev is not None:
                        pv(*prev)
                    prev = (qc, kt, ex)
                pv(*prev)
                for f in eq:
                    f()
                for fl in fq:
                    fl.fn()
                return norm_closures(1)

            # ---- schedule ----
            load_xt(0)
            alloc_v(0)
            qk00 = qkgen_fills(0, 0)
            for fl in qk00:
                fl.fn()

            # fills per unit: qkgen of the next unit everywhere, plus
            # vgen(next batch) on (b,2..5) and proj(prev batch) on (b,0..3).
            earlies = []
            norm_last = None
            for b in range(B):
                ao = p_ao.tile([128, DT, S], BF16, tag="ao")
                aos[b] = ao
                for hp in range(HP):
                    fills = []
                    late = []
                    min_rate = 0.0
                    if hp == 1 and b + 1 < B:
                        load_xt(b + 1)
                        alloc_v(b + 1)
                    if b == 0 and hp == 0:
                        fills += vgen_fills(0)
                        min_rate = 13.0
                    if not (b == B - 1 and hp == HP - 1):
                        nb, nhp = (b, hp + 1) if hp + 1 < HP else (b + 1, 0)
                        fills += qkgen_fills(nb, nhp)
                    if b + 1 < B and hp >= 1:
                        vg = vgen_fills(b + 1) if hp == 1 else vgs
                        vgs = vg
                        lo, hi = [(0, 3), (3, 6), (6, 9), (9, 12),
                                  (12, 16)][hp - 1]
                        fills += vg[lo:hi]
                    if b > 0 and hp <= 4:
                        tts = [(0, 1), (2, 3), (4, 5), (6,), (7,)][hp]
                        for t in tts:
                            fills += proj_fills_tt(b - 1, t)
                    if b == B - 1 and hp == HP - 1:
                        for tt in range(4):
                            late += proj_fills_tt(b, tt)
                        for tt in range(4, TT):
                            late += proj_fills_tt(b, tt, defer_j5=True)
                    norm_last = unit(b, hp, fills, earlies,
                                     min_rate=min_rate, late_fills=late)
                    earlies = norm_last
            for f in earlies:
                f()
            for tt in range(4, TT):
                proj_finish_tt(B - 1, tt)
    nc.finalize()
    return nc


def _marshal(x, W_qkv, W_out, b_out):
    bf = ml_dtypes.bfloat16
    wqkvT = np.ascontiguousarray(W_qkv.T).astype(bf)
    woutT = np.ascontiguousarray(W_out.T).astype(bf)
    bfull = np.ascontiguousarray(np.broadcast_to(
        np.asarray(b_out, np.float32).reshape(1, D), (128, D)))
    in_maps = []
    for c in range(N_CORES):
        xc = np.ascontiguousarray(
            np.asarray(x)[B * c:B * (c + 1)].reshape(B * S, D).T
        ).astype(bf)
        in_maps.append({
            "xT": xc, "wqkvT": wqkvT, "woutT": woutT, "biasf": bfull,
        })
    return in_maps


def run(x, W_qkv, W_out, b_out, trace=False, **spmd_kwargs):
    if "nc" not in _CACHE:
        _CACHE["nc"] = build_nc()
    nc = _CACHE["nc"]
    in_maps = _marshal(x, W_qkv, W_out, b_out)
    res = run_bass_kernel_spmd(
        nc, in_maps, core_ids=list(range(N_CORES)), trace=trace, **spmd_kwargs
    )
    out = np.stack([res.results[c]["y"] for c in range(N_CORES)], axis=0)
    out = out.reshape(N_CORES * B, S, D)
    return out, res


def kernel(x, W_qkv, W_out, b_out):
    out, _ = run(x, W_qkv, W_out, b_out)
    return out


# revision 7
# speedup vs baseline: 1.0113x; 1.0032x over previous
"""Multi-head attention (B=16, S=1024, D=768, H=12) on 8 TRN2 NeuronCores.

Strategy: pure data parallelism — batch 16 is split 2-per-core; weights are
replicated. Each core runs an identical Bass/Tile program on its own x shard.

Optimizations vs the 438us v1 baseline (now ~331us):
  - all matmul inputs bf16 (x, W_qkv, q/k tiles): enables FWL weight loads,
    halves input DMA. PE accumulates fp32 so scores/out stay accurate.
  - every unit gets "fill" matmul work (qkgen of the NEXT unit, vgen of the
    next batch, proj of the previous batch), paced by an adaptive per-step
    budget and emitted BETWEEN the scores pair and the exp-dependent PV
    matmuls, so the in-order PE never stalls at the queue head waiting on
    ACT. This keeps PE duty high so the HAM clock gate stays at 2.4 GHz.
  - normalize split per-qc: qc0 normalization of a unit runs inside the same
    unit (step 9+), qc1 drips into the next unit. The last unit overlaps the
    final batch's qc0 projection; tt4-7 accumulate d-tiles 0..4 early and
    only the j=5 slice + bias-add run in the tail (proj_finish_tt).
  - reciprocal -> reciprocal_approx_fast (~5x faster, 18-bit accurate;
    NaNs if its APs start above partition 0 - keep denom tiles base-0).
  - bias via DVE tensor_add of a pre-broadcast [128,D] bias tile (no K=1
    bias matmuls), weight DMAs merged and spread across the scalar/gpsimd
    queues so the x shard + first head-pair's q/k land first.

Measurement note: runs intermittently land in a ~1.19x chip-wide slow
mode (all engines inflate, including fixed-clock ACT) — re-run before
judging any variant. Mode-adjusted, padding the PV stationary operand to
128 weight columns was still ~+7us worse (extra LDWEIGHTS columns); fill
granularity, step pair-batching, weight-sharing, DMA queue fan-out, and
contiguous x marshaling all measured neutral — the ~53us of per-matmul
friction above the 1-col/cycle streaming floor is schedule-invariant.

Per-core program (b in 0..1, head-pairs hp in 0..5):
  - v  = x @ W_v^T           [t, e] head-interleaved + ones col -> PV lhsT
  - qT2/kT2 [128, S]         two heads stacked on partitions (d-major)
  - scoresT[k,q] = k q^T     row-packed per head via tile_position (K=64)
  - exp on ACT (scale=1/8) -> bf16 SBUF tile
  - PV: out[dh+1, q] += v_ext.T @ exp   (row 64 accumulates the denom)
  - normalize: denom rows staged at partitions 0/32/64/96, approx-reciprocal
    per qc half, gpsimd partition_broadcast, DVE mult -> attn_outT (bf16)
  - y = attn_outT.T @ W_out^T + b_out  (bias added on DVE)
"""
import ml_dtypes
import numpy as np
import concourse.bacc as bacc
import concourse.tile as tile
from concourse import mybir
from concourse.bass_utils import run_bass_kernel_spmd

FP32 = mybir.dt.float32
BF16 = mybir.dt.bfloat16
EXP = mybir.ActivationFunctionType.Exp

B, S, D, H = 2, 1024, 768, 12       # per-core batch of 2
HP = H // 2                          # head pairs (6)
DT = D // 128                        # d tiles (6)
KT = S // 128                        # k tiles (8)
QC = S // 512                        # q chunks (2)
TT = S // 128                        # t tiles per batch (8)
N_CORES = 8

_CACHE = {}


class Fill:
    __slots__ = ("fn", "cost")

    def __init__(self, fn, cost):
        self.fn = fn
        self.cost = cost


def build_nc():
    nc = bacc.Bacc(trn_type="TRN2")
    xT = nc.dram_tensor("xT", [D, B * S], BF16, kind="ExternalInput")
    wqkvT = nc.dram_tensor("wqkvT", [D, 3 * D], BF16, kind="ExternalInput")
    woutT = nc.dram_tensor("woutT", [D, D], BF16, kind="ExternalInput")
    biasf = nc.dram_tensor("biasf", [128, D], FP32, kind="ExternalInput")
    y = nc.dram_tensor("y", [B * S, D], FP32, kind="ExternalOutput")

    with tile.TileContext(nc) as tc:
        with (
            tc.tile_pool(name="wq", bufs=1) as p_wq,
            tc.tile_pool(name="wo", bufs=1) as p_wo,
            tc.tile_pool(name="cst", bufs=1) as p_cst,
            tc.tile_pool(name="wrm", bufs=1) as p_warm,
            tc.tile_pool(name="xt", bufs=2) as p_xt,
            tc.tile_pool(name="vv", bufs=2) as p_v,
            tc.tile_pool(name="ao", bufs=2) as p_ao,
            tc.tile_pool(name="qk", bufs=4) as p_qk,
            tc.tile_pool(name="exp", bufs=3) as p_exp,
            tc.tile_pool(name="oc", bufs=6) as p_oc,
            tc.tile_pool(name="dn", bufs=2) as p_dn,
            tc.tile_pool(name="yy", bufs=8) as p_y,
            tc.tile_pool(name="rb", bufs=2) as p_rb,
            tc.tile_pool(name="r0", bufs=2) as p_r0,
            tc.tile_pool(name="sc", bufs=2, space="PSUM") as p_sc,
            tc.tile_pool(name="gen", bufs=2, space="PSUM") as p_gen,
            tc.tile_pool(name="oacc", bufs=2, space="PSUM") as p_oacc,
        ):
            wq = p_wq.tile([128, DT, 3 * D], BF16)
            wo = p_wo.tile([128, DT, D], BF16)
            bias_t = p_cst.tile([128, D], FP32)
            wqr = wqkvT.rearrange("(j p) e -> p j e", p=128)
            wor = woutT.rearrange("(j p) e -> p j e", p=128)
            # parallel queues: hp0 q/k + rest on scalar, v-cols on vector,
            # W_out + bias on gpsimd, x on sync (in load_xt below)
            nc.scalar.dma_start(wq[:, :, 0:128], wqr[:, :, 0:128])
            nc.scalar.dma_start(wq[:, :, D:D + 128], wqr[:, :, D:D + 128])
            nc.scalar.dma_start(wq[:, :, 2 * D:3 * D], wqr[:, :, 2 * D:3 * D])
            nc.scalar.dma_start(wq[:, :, 128:D], wqr[:, :, 128:D])
            nc.scalar.dma_start(
                wq[:, :, D + 128:2 * D], wqr[:, :, D + 128:2 * D])
            nc.scalar.dma_start(wo[:, :, :], wor[:, :, :])
            nc.gpsimd.dma_start(bias_t[:], biasf[:])

            xts, vs, aos = {}, {}, {}

            def load_xt(b):
                xt = p_xt.tile([128, DT, S], BF16, tag="xt")
                for j in range(DT):
                    nc.sync.dma_start(
                        xt[:, j, :], xT[128 * j:128 * (j + 1), b * S:(b + 1) * S]
                    )
                xts[b] = xt

            def alloc_v(b):
                v = p_v.tile([128, KT, H, 65], BF16, tag="vv")
                nc.vector.memset(v[:, :, :, 64], 1.0)
                vs[b] = v

            def vgen_fills(b):
                """16 fills: one [128,512-or-256] psum group + copy each."""
                fills = []
                for tt in range(TT):
                    for h0, nh in ((0, 8), (8, 4)):
                        def f(tt=tt, h0=h0, nh=nh, b=b):
                            xt, v = xts[b], vs[b]
                            vp = p_gen.tile([128, 512], FP32, tag="gen")
                            cw = nh * 64
                            for j in range(DT):
                                nc.tensor.matmul(
                                    vp[:, 0:cw],
                                    xt[:, j, tt * 128:(tt + 1) * 128],
                                    wq[:, j,
                                       2 * D + h0 * 64:2 * D + h0 * 64 + cw],
                                    start=(j == 0), stop=(j == DT - 1),
                                )
                            nc.vector.tensor_copy(
                                v[:, tt, h0:h0 + nh, 0:64],
                                vp[:, 0:cw].rearrange("p (h c) -> p h c", h=nh),
                            )
                        fills.append(Fill(f, 6))
                return fills

            proj_parts = {}

            def proj_fills_tt(b, tt, defer_j5=False):
                """2 fills for one token tile: y(b, tt) projection. With
                defer_j5, only d-tiles 0..4 accumulate (the j=5 slice of ao
                isn't normalized yet); proj_finish_tt adds the rest."""
                fills = []
                box = {}
                nj = DT - 1 if defer_j5 else DT
                for ci, (c0, cw) in enumerate(((0, 512), (512, 256))):
                    def f(tt=tt, ci=ci, c0=c0, cw=cw, b=b, box=box, nj=nj,
                          defer_j5=defer_j5):
                        ao = aos[b]
                        if ci == 0:
                            ys = p_y.tile([128, D], FP32, tag="yy")
                            box["ys"] = ys
                            proj_parts[(b, tt)] = ys
                        ys = box["ys"]
                        yp = p_gen.tile([128, 512], FP32, tag="gen")
                        for j in range(nj):
                            nc.tensor.matmul(
                                yp[:, 0:cw],
                                ao[:, j, tt * 128:(tt + 1) * 128],
                                wo[:, j, c0:c0 + cw],
                                start=(j == 0), stop=(j == nj - 1),
                            )
                        nc.vector.tensor_add(
                            ys[:, c0:c0 + cw], yp[:, 0:cw],
                            bias_t[:, c0:c0 + cw],
                        )
                        if ci == 1 and not defer_j5:
                            nc.sync.dma_start(
                                y[b * S + tt * 128:b * S + (tt + 1) * 128, :],
                                ys[:],
                            )
                    fills.append(Fill(f, nj))
                return fills

            def proj_finish_tt(b, tt):
                """Add the deferred j=5 contribution into the partial ys."""
                ao, ys = aos[b], proj_parts[(b, tt)]
                for c0, cw in ((0, 512), (512, 256)):
                    yp = p_gen.tile([128, 512], FP32, tag="gen")
                    nc.tensor.matmul(
                        yp[:, 0:cw],
                        ao[:, DT - 1, tt * 128:(tt + 1) * 128],
                        wo[:, DT - 1, c0:c0 + cw],
                        start=True, stop=True,
                    )
                    nc.vector.tensor_add(
                        ys[:, c0:c0 + cw], yp[:, 0:cw], ys[:, c0:c0 + cw])
                nc.sync.dma_start(
                    y[b * S + tt * 128:b * S + (tt + 1) * 128, :], ys[:])

            qkts = {}

            def qkgen_fills(b, hp):
                """4 fills building unit (b,hp)'s qT2/kT2 [128, S] tiles."""
                sqs = [
                    p_qk.tile([128, S], BF16, tag="qk", name=f"qk{b}_{hp}_{i}")
                    for i in range(2)
                ]
                qkts[(b, hp)] = sqs
                fills = []
                for part in range(2):  # 0 = q, 1 = k
                    for qc in range(QC):
                        def f(part=part, qc=qc, b=b, hp=hp):
                            qp = p_gen.tile([128, 512], FP32, tag="gen")
                            for j in range(DT):
                                nc.tensor.matmul(
                                    qp[:, :],
                                    wq[:, j,
                                       part * D + 128 * hp:part * D + 128 * (hp + 1)],
                                    xts[b][:, j, qc * 512:(qc + 1) * 512],
                                    start=(j == 0), stop=(j == DT - 1),
                                )
                            nc.vector.tensor_copy(
                                sqs[part][:, qc * 512:(qc + 1) * 512], qp[:, :]
                            )
                        fills.append(Fill(f, 6))
                return fills

            def unit(b, hp, fills, earlies, min_rate=0.0, late_fills=()):
                """One (batch, head-pair) attention unit: 16 scores/exp/PV
                steps with fill work dripped between the scores pair and the
                exp-dependent PV so the in-order PE never idles at the queue
                head. Returns the qc1-normalize closures (earlies for the
                next unit)."""
                v, ao = vs[b], aos[b]
                qT2, kT2 = qkts[(b, hp)]

                ocs, oaccs = {}, {}
                dns, dnrs = {}, {}

                def scores_exp(qc, kt):
                    sc = p_sc.tile([128, 1024], FP32, tag="sc")
                    nc.tensor.matmul(
                        sc[:, 0:512],
                        kT2[0:64, kt * 128:(kt + 1) * 128],
                        qT2[0:64, qc * 512:(qc + 1) * 512],
                        start=True, stop=True, tile_position=(0, 0),
                    )
                    nc.tensor.matmul(
                        sc[:, 512:1024],
                        kT2[64:128, kt * 128:(kt + 1) * 128],
                        qT2[64:128, qc * 512:(qc + 1) * 512],
                        start=True, stop=True, tile_position=(64, 0),
                    )
                    ex = p_exp.tile([128, 1024], BF16, tag="exp")
                    nc.scalar.activation(ex[:], sc[:], EXP, scale=0.125)
                    return ex

                def pv(qc, kt, ex):
                    if kt == 0:
                        o_a = p_oacc.tile([65, 512], FP32, tag="oacc")
                        o_b = p_oacc.tile([65, 512], FP32, tag="oacc")
                        oaccs[(qc, 0)] = o_a
                        oaccs[(qc, 1)] = o_b
                    nc.tensor.matmul(
                        oaccs[(qc, 0)][:], v[:, kt, 2 * hp, :], ex[:, 0:512],
                        start=(kt == 0), stop=(kt == KT - 1),
                    )
                    nc.tensor.matmul(
                        oaccs[(qc, 1)][:], v[:, kt, 2 * hp + 1, :],
                        ex[:, 512:1024],
                        start=(kt == 0), stop=(kt == KT - 1),
                    )
                    if kt == KT - 1:
                        # denom rows staged into a base-partition-0 tile
                        # (reciprocal_approx_fast NaNs on base-partition>0)
                        dn = p_dn.tile([64, 512], FP32, tag="dn")
                        nc.vector.memset(dn[:], 1.0)
                        dns[qc] = dn
                        for head in range(2):
                            oc = p_oc.tile([65, 512], FP32, tag="oc")
                            nc.vector.tensor_copy(oc[:], oaccs[(qc, head)][:])
                            nc.vector.tensor_copy(
                                dn[32 * head:32 * head + 1, :], oc[64:65, :]
                            )
                            ocs[(qc, head)] = oc

                rbs = {}

                def norm_closures(qc):
                    """5 closures: recip, then per head a (copy+broadcast)
                    stage and a mult stage (split so the DVE isn't parked
                    behind the 1.2us gpsimd broadcast)."""
                    cls = []
                    def recip(qc=qc):
                        dnr = p_dn.tile([64, 512], FP32, tag="dnr")
                        dnrs[qc] = dnr
                        nc.vector.reciprocal_approx_fast(
                            out=dnr[:], in_=dns[qc][:],
                        )
                    cls.append(recip)
                    for head in range(2):
                        def f1(qc=qc, head=head):
                            if head == 0:
                                src_row = dnrs[qc][0:1, :]
                            else:
                                r0 = p_r0.tile([1, 512], FP32, tag="r0")
                                nc.vector.tensor_copy(
                                    r0[:], dnrs[qc][32:33, :])
                                src_row = r0[:]
                            rb = p_rb.tile([64, 512], FP32, tag="rb")
                            nc.gpsimd.partition_broadcast(rb[:], src_row)
                            rbs[(qc, head)] = rb
                        def f2(qc=qc, head=head):
                            nc.vector.tensor_mul(
                                ao[64 * head:64 * (head + 1), hp,
                                   qc * 512:(qc + 1) * 512],
                                ocs[(qc, head)][0:64, :], rbs[(qc, head)],
                            )
                        cls.append((f1, f2))
                    (a1, a2), (b1, b2) = cls[1], cls[2]
                    return [cls[0], a1, b1, a2, b2]

                eq = list(earlies)
                fq = list(fills)
                debt = 0.0
                prev = None
                for step in range(QC * KT):
                    qc, kt = divmod(step, KT)
                    ex = scores_exp(qc, kt)
                    if step == 9:
                        # qc0 of this unit is fully PV'd (step 8): normalize
                        # it in-unit, then any late fills (kept FIFO-after
                        # the norm closures they depend on)
                        fq.extend(Fill(c, 0.5) for c in norm_closures(0))
                        fq.extend(late_fills)
                    if eq:
                        eq.pop(0)()
                    steps_left = QC * KT - step
                    total_left = sum(fl.cost for fl in fq)
                    debt = min(debt + max(total_left / steps_left, min_rate),
                               14.0)
                    while fq and fq[0].cost <= debt:
                        fl = fq.pop(0)
                        debt -= fl.cost
                        fl.fn()
                    if prev is not None:
                        pv(*prev)
                    prev = (qc, kt, ex)
                pv(*prev)
                for f in eq:
                    f()
                for fl in fq:
                    fl.fn()
                return norm_closures(1)

            # warm the HAM clock gate during the DMA-bound head: ~3.4us
            # of dummy matmuls in the otherwise-idle 6.5-11us window so the
            # first real matmuls run at 2.4GHz instead of 1.2
            wrm = p_warm.tile([128, 512], BF16)
            nc.vector.memset(wrm[:], 1.0)
            for i in range(8):
                wp = p_gen.tile([128, 512], FP32, tag="gen",
                                name=f"warm{i}")
                nc.tensor.matmul(wp[:], wrm[:, 0:128], wrm[:, :],
                                 start=True, stop=True)

            # ---- schedule ----
            load_xt(0)
            alloc_v(0)
            qk00 = qkgen_fills(0, 0)
            for fl in qk00:
                fl.fn()

            # fills per unit: qkgen of the next unit everywhere, plus
            # vgen(next batch) on (b,2..5) and proj(prev batch) on (b,0..3).
            earlies = []
            norm_last = None
            for b in range(B):
                ao = p_ao.tile([128, DT, S], BF16, tag="ao")
                aos[b] = ao
                for hp in range(HP):
                    fills = []
                    late = []
                    min_rate = 0.0
                    if hp == 1 and b + 1 < B:
                        load_xt(b + 1)
                        alloc_v(b + 1)
                    if b == 0 and hp == 0:
                        fills += vgen_fills(0)
                        min_rate = 13.0
                    if not (b == B - 1 and hp == HP - 1):
                        nb, nhp = (b, hp + 1) if hp + 1 < HP else (b + 1, 0)
                        fills += qkgen_fills(nb, nhp)
                    if b + 1 < B and hp >= 1:
                        vg = vgen_fills(b + 1) if hp == 1 else vgs
                        vgs = vg
                        lo, hi = [(0, 3), (3, 6), (6, 9), (9, 12),
                                  (12, 16)][hp - 1]
                        fills += vg[lo:hi]
                    if b > 0 and hp <= 4:
                        tts = [(0, 1), (2, 3), (4, 5), (6,), (7,)][hp]
                        for t in tts:
                            fills += proj_fills_tt(b - 1, t)
                    if b == B - 1 and hp == HP - 1:
                        for tt in range(4):
                            late += proj_fills_tt(b, tt)
                        for tt in range(4, TT):
                            late += proj_fills_tt(b, tt, defer_j5=True)
                    norm_last = unit(b, hp, fills, earlies,
                                     min_rate=min_rate, late_fills=late)
                    earlies = norm_last
            for f in earlies:
                f()
            for tt in range(4, TT):
                proj_finish_tt(B - 1, tt)
    nc.finalize()
    return nc


def _marshal(x, W_qkv, W_out, b_out):
    bf = ml_dtypes.bfloat16
    wqkvT = np.ascontiguousarray(W_qkv.T).astype(bf)
    woutT = np.ascontiguousarray(W_out.T).astype(bf)
    bfull = np.ascontiguousarray(np.broadcast_to(
        np.asarray(b_out, np.float32).reshape(1, D), (128, D)))
    in_maps = []
    for c in range(N_CORES):
        xc = np.ascontiguousarray(
            np.asarray(x)[B * c:B * (c + 1)].reshape(B * S, D).T
        ).astype(bf)
        in_maps.append({
            "xT": xc, "wqkvT": wqkvT, "woutT": woutT, "biasf": bfull,
        })
    return in_maps


def run(x, W_qkv, W_out, b_out, trace=False, **spmd_kwargs):
    if "nc" not in _CACHE:
        _CACHE["nc"] = build_nc()
    nc = _CACHE["nc"]
    in_maps = _marshal(x, W_qkv, W_out, b_out)
    res = run_bass_kernel_spmd(
        nc, in_maps, core_ids=list(range(N_CORES)), trace=trace, **spmd_kwargs
    )
    out = np.stack([res.results[c]["y"] for c in range(N_CORES)], axis=0)
    out = out.reshape(N_CORES * B, S, D)
    return out, res


def kernel(x, W_qkv, W_out, b_out):
    out, _ = run(x, W_qkv, W_out, b_out)
    return out


# revision 8
# speedup vs baseline: 1.0116x; 1.0003x over previous
"""Multi-head attention (B=16, S=1024, D=768, H=12) on 8 TRN2 NeuronCores.

Strategy: pure data parallelism — batch 16 is split 2-per-core; weights are
replicated. Each core runs an identical Bass/Tile program on its own x shard.

Optimizations vs the 438us v1 baseline (now ~331us):
  - all matmul inputs bf16 (x, W_qkv, q/k tiles): enables FWL weight loads,
    halves input DMA. PE accumulates fp32 so scores/out stay accurate.
  - every unit gets "fill" matmul work (qkgen of the NEXT unit, vgen of the
    next batch, proj of the previous batch), paced by an adaptive per-step
    budget and emitted BETWEEN the scores pair and the exp-dependent PV
    matmuls, so the in-order PE never stalls at the queue head waiting on
    ACT. This keeps PE duty high so the HAM clock gate stays at 2.4 GHz.
  - normalize split per-qc: qc0 normalization of a unit runs inside the same
    unit (step 9+), qc1 drips into the next unit. The last unit overlaps the
    final batch's qc0 projection; tt4-7 accumulate d-tiles 0..4 early and
    only the j=5 slice + bias-add run in the tail (proj_finish_tt).
  - reciprocal -> reciprocal_approx_fast (~5x faster, 18-bit accurate;
    NaNs if its APs start above partition 0 - keep denom tiles base-0).
  - bias via DVE tensor_add of a pre-broadcast [128,D] bias tile (no K=1
    bias matmuls), weight DMAs merged and spread across the scalar/gpsimd
    queues so the x shard + first head-pair's q/k land first.

Measurement note: runs intermittently land in a ~1.19x chip-wide slow
mode (all engines inflate, including fixed-clock ACT) — re-run before
judging any variant. Mode-adjusted, padding the PV stationary operand to
128 weight columns was still ~+7us worse (extra LDWEIGHTS columns); fill
granularity, step pair-batching, weight-sharing, DMA queue fan-out, and
contiguous x marshaling all measured neutral — the ~53us of per-matmul
friction above the 1-col/cycle streaming floor is schedule-invariant.

Per-core program (b in 0..1, head-pairs hp in 0..5):
  - v  = x @ W_v^T           [t, e] head-interleaved + ones col -> PV lhsT
  - qT2/kT2 [128, S]         two heads stacked on partitions (d-major)
  - scoresT[k,q] = k q^T     row-packed per head via tile_position (K=64)
  - exp on ACT (scale=1/8) -> bf16 SBUF tile
  - PV: out[dh+1, q] += v_ext.T @ exp   (row 64 accumulates the denom)
  - normalize: denom rows staged at partitions 0/32/64/96, approx-reciprocal
    per qc half, gpsimd partition_broadcast, DVE mult -> attn_outT (bf16)
  - y = attn_outT.T @ W_out^T + b_out  (bias added on DVE)
"""
import ml_dtypes
import numpy as np
import concourse.bacc as bacc
import concourse.tile as tile
from concourse import mybir
from concourse.bass_utils import run_bass_kernel_spmd

FP32 = mybir.dt.float32
BF16 = mybir.dt.bfloat16
EXP = mybir.ActivationFunctionType.Exp

B, S, D, H = 2, 1024, 768, 12       # per-core batch of 2
HP = H // 2                          # head pairs (6)
DT = D // 128                        # d tiles (6)
KT = S // 128                        # k tiles (8)
QC = S // 512                        # q chunks (2)
TT = S // 128                        # t tiles per batch (8)
N_CORES = 8

_CACHE = {}


class Fill:
    __slots__ = ("fn", "cost")

    def __init__(self, fn, cost):
        self.fn = fn
        self.cost = cost


def build_nc():
    nc = bacc.Bacc(trn_type="TRN2")
    xT = nc.dram_tensor("xT", [D, B * S], BF16, kind="ExternalInput")
    wqkvT = nc.dram_tensor("wqkvT", [D, 3 * D], BF16, kind="ExternalInput")
    woutT = nc.dram_tensor("woutT", [D, D], BF16, kind="ExternalInput")
    biasf = nc.dram_tensor("biasf", [128, D], FP32, kind="ExternalInput")
    y = nc.dram_tensor("y", [B * S, D], FP32, kind="ExternalOutput")

    with tile.TileContext(nc) as tc:
        with (
            tc.tile_pool(name="wq", bufs=1) as p_wq,
            tc.tile_pool(name="wo", bufs=1) as p_wo,
            tc.tile_pool(name="cst", bufs=1) as p_cst,
            tc.tile_pool(name="wrm", bufs=1) as p_warm,
            tc.tile_pool(name="xt", bufs=2) as p_xt,
            tc.tile_pool(name="vv", bufs=2) as p_v,
            tc.tile_pool(name="ao", bufs=2) as p_ao,
            tc.tile_pool(name="qk", bufs=4) as p_qk,
            tc.tile_pool(name="exp", bufs=3) as p_exp,
            tc.tile_pool(name="oc", bufs=6) as p_oc,
            tc.tile_pool(name="dn", bufs=2) as p_dn,
            tc.tile_pool(name="yy", bufs=8) as p_y,
            tc.tile_pool(name="rb", bufs=2) as p_rb,
            tc.tile_pool(name="r0", bufs=2) as p_r0,
            tc.tile_pool(name="sc", bufs=2, space="PSUM") as p_sc,
            tc.tile_pool(name="gen", bufs=2, space="PSUM") as p_gen,
            tc.tile_pool(name="oacc", bufs=2, space="PSUM") as p_oacc,
        ):
            wq = p_wq.tile([128, DT, 3 * D], BF16)
            wo = p_wo.tile([128, DT, D], BF16)
            bias_t = p_cst.tile([128, D], FP32)
            wqr = wqkvT.rearrange("(j p) e -> p j e", p=128)
            wor = woutT.rearrange("(j p) e -> p j e", p=128)
            # parallel queues: hp0 q/k + rest on scalar, v-cols on vector,
            # W_out + bias on gpsimd, x on sync (in load_xt below)
            nc.scalar.dma_start(wq[:, :, 0:128], wqr[:, :, 0:128])
            nc.scalar.dma_start(wq[:, :, D:D + 128], wqr[:, :, D:D + 128])
            nc.scalar.dma_start(wq[:, :, 2 * D:3 * D], wqr[:, :, 2 * D:3 * D])
            nc.scalar.dma_start(wq[:, :, 128:D], wqr[:, :, 128:D])
            nc.scalar.dma_start(
                wq[:, :, D + 128:2 * D], wqr[:, :, D + 128:2 * D])
            nc.scalar.dma_start(wo[:, :, :], wor[:, :, :])
            nc.gpsimd.dma_start(bias_t[:], biasf[:])

            xts, vs, aos = {}, {}, {}

            def load_xt(b):
                xt = p_xt.tile([128, DT, S], BF16, tag="xt")
                for j in range(DT):
                    nc.sync.dma_start(
                        xt[:, j, :], xT[128 * j:128 * (j + 1), b * S:(b + 1) * S]
                    )
                xts[b] = xt

            def alloc_v(b):
                v = p_v.tile([128, KT, H, 65], BF16, tag="vv")
                nc.vector.memset(v[:, :, :, 64], 1.0)
                vs[b] = v

            def vgen_fills(b):
                """16 fills: one [128,512-or-256] psum group + copy each."""
                fills = []
                for tt in range(TT):
                    for h0, nh in ((0, 8), (8, 4)):
                        def f(tt=tt, h0=h0, nh=nh, b=b):
                            xt, v = xts[b], vs[b]
                            vp = p_gen.tile([128, 512], FP32, tag="gen")
                            cw = nh * 64
                            for j in range(DT):
                                nc.tensor.matmul(
                                    vp[:, 0:cw],
                                    xt[:, j, tt * 128:(tt + 1) * 128],
                                    wq[:, j,
                                       2 * D + h0 * 64:2 * D + h0 * 64 + cw],
                                    start=(j == 0), stop=(j == DT - 1),
                                )
                            nc.vector.tensor_copy(
                                v[:, tt, h0:h0 + nh, 0:64],
                                vp[:, 0:cw].rearrange("p (h c) -> p h c", h=nh),
                            )
                        fills.append(Fill(f, 6))
                return fills

            proj_parts = {}

            def proj_fills_tt(b, tt, defer_j5=False):
                """2 fills for one token tile: y(b, tt) projection. With
                defer_j5, only d-tiles 0..4 accumulate (the j=5 slice of ao
                isn't normalized yet); proj_finish_tt adds the rest."""
                fills = []
                box = {}
                nj = DT - 1 if defer_j5 else DT
                for ci, (c0, cw) in enumerate(((0, 512), (512, 256))):
                    def f(tt=tt, ci=ci, c0=c0, cw=cw, b=b, box=box, nj=nj,
                          defer_j5=defer_j5):
                        ao = aos[b]
                        if ci == 0:
                            ys = p_y.tile([128, D], FP32, tag="yy")
                            box["ys"] = ys
                            proj_parts[(b, tt)] = ys
                        ys = box["ys"]
                        yp = p_gen.tile([128, 512], FP32, tag="gen")
                        for j in range(nj):
                            nc.tensor.matmul(
                                yp[:, 0:cw],
                                ao[:, j, tt * 128:(tt + 1) * 128],
                                wo[:, j, c0:c0 + cw],
                                start=(j == 0), stop=(j == nj - 1),
                            )
                        nc.vector.tensor_add(
                            ys[:, c0:c0 + cw], yp[:, 0:cw],
                            bias_t[:, c0:c0 + cw],
                        )
                        if ci == 1 and not defer_j5:
                            nc.sync.dma_start(
                                y[b * S + tt * 128:b * S + (tt + 1) * 128, :],
                                ys[:],
                            )
                    fills.append(Fill(f, nj))
                return fills

            def proj_finish_tt(b, tt):
                """Add the deferred j=5 contribution into the partial ys."""
                ao, ys = aos[b], proj_parts[(b, tt)]
                for c0, cw in ((0, 512), (512, 256)):
                    yp = p_gen.tile([128, 512], FP32, tag="gen")
                    nc.tensor.matmul(
                        yp[:, 0:cw],
                        ao[:, DT - 1, tt * 128:(tt + 1) * 128],
                        wo[:, DT - 1, c0:c0 + cw],
                        start=True, stop=True,
                    )
                    nc.vector.tensor_add(
                        ys[:, c0:c0 + cw], yp[:, 0:cw], ys[:, c0:c0 + cw])
                nc.sync.dma_start(
                    y[b * S + tt * 128:b * S + (tt + 1) * 128, :], ys[:])

            qkts = {}

            def qkgen_fills(b, hp):
                """4 fills building unit (b,hp)'s qT2/kT2 [128, S] tiles."""
                sqs = [
                    p_qk.tile([128, S], BF16, tag="qk", name=f"qk{b}_{hp}_{i}")
                    for i in range(2)
                ]
                qkts[(b, hp)] = sqs
                fills = []
                for part in range(2):  # 0 = q, 1 = k
                    for qc in range(QC):
                        def f(part=part, qc=qc, b=b, hp=hp):
                            qp = p_gen.tile([128, 512], FP32, tag="gen")
                            for j in range(DT):
                                nc.tensor.matmul(
                                    qp[:, :],
                                    wq[:, j,
                                       part * D + 128 * hp:part * D + 128 * (hp + 1)],
                                    xts[b][:, j, qc * 512:(qc + 1) * 512],
                                    start=(j == 0), stop=(j == DT - 1),
                                )
                            nc.vector.tensor_copy(
                                sqs[part][:, qc * 512:(qc + 1) * 512], qp[:, :]
                            )
                        fills.append(Fill(f, 6))
                return fills

            def unit(b, hp, fills, earlies, min_rate=0.0, late_fills=()):
                """One (batch, head-pair) attention unit: 16 scores/exp/PV
                steps with fill work dripped between the scores pair and the
                exp-dependent PV so the in-order PE never idles at the queue
                head. Returns the qc1-normalize closures (earlies for the
                next unit)."""
                v, ao = vs[b], aos[b]
                qT2, kT2 = qkts[(b, hp)]

                ocs, oaccs = {}, {}
                dns, dnrs = {}, {}

                def scores_exp(qc, kt):
                    sc = p_sc.tile([128, 1024], FP32, tag="sc")
                    nc.tensor.matmul(
                        sc[:, 0:512],
                        kT2[0:64, kt * 128:(kt + 1) * 128],
                        qT2[0:64, qc * 512:(qc + 1) * 512],
                        start=True, stop=True, tile_position=(0, 0),
                    )
                    nc.tensor.matmul(
                        sc[:, 512:1024],
                        kT2[64:128, kt * 128:(kt + 1) * 128],
                        qT2[64:128, qc * 512:(qc + 1) * 512],
                        start=True, stop=True, tile_position=(64, 0),
                    )
                    ex = p_exp.tile([128, 1024], BF16, tag="exp")
                    nc.scalar.activation(ex[:], sc[:], EXP, scale=0.125)
                    return ex

                def pv(qc, kt, ex):
                    if kt == 0:
                        o_a = p_oacc.tile([65, 512], FP32, tag="oacc")
                        o_b = p_oacc.tile([65, 512], FP32, tag="oacc")
                        oaccs[(qc, 0)] = o_a
                        oaccs[(qc, 1)] = o_b
                    nc.tensor.matmul(
                        oaccs[(qc, 0)][:], v[:, kt, 2 * hp, :], ex[:, 0:512],
                        start=(kt == 0), stop=(kt == KT - 1),
                    )
                    nc.tensor.matmul(
                        oaccs[(qc, 1)][:], v[:, kt, 2 * hp + 1, :],
                        ex[:, 512:1024],
                        start=(kt == 0), stop=(kt == KT - 1),
                    )
                    if kt == KT - 1:
                        # denom rows staged into a base-partition-0 tile
                        # (reciprocal_approx_fast NaNs on base-partition>0)
                        dn = p_dn.tile([64, 512], FP32, tag="dn")
                        nc.vector.memset(dn[:], 1.0)
                        dns[qc] = dn
                        for head in range(2):
                            oc = p_oc.tile([65, 512], FP32, tag="oc")
                            nc.vector.tensor_copy(oc[:], oaccs[(qc, head)][:])
                            nc.vector.tensor_copy(
                                dn[32 * head:32 * head + 1, :], oc[64:65, :]
                            )
                            ocs[(qc, head)] = oc

                rbs = {}

                def norm_closures(qc):
                    """5 closures: recip, then per head a (copy+broadcast)
                    stage and a mult stage (split so the DVE isn't parked
                    behind the 1.2us gpsimd broadcast)."""
                    cls = []
                    def recip(qc=qc):
                        dnr = p_dn.tile([64, 512], FP32, tag="dnr")
                        dnrs[qc] = dnr
                        nc.vector.reciprocal_approx_fast(
                            out=dnr[:], in_=dns[qc][:],
                        )
                    cls.append(recip)
                    for head in range(2):
                        def f1(qc=qc, head=head):
                            if head == 0:
                                src_row = dnrs[qc][0:1, :]
                            else:
                                r0 = p_r0.tile([1, 512], FP32, tag="r0")
                                nc.vector.tensor_copy(
                                    r0[:], dnrs[qc][32:33, :])
                                src_row = r0[:]
                            rb = p_rb.tile([64, 512], FP32, tag="rb")
                            nc.gpsimd.partition_broadcast(rb[:], src_row)
                            rbs[(qc, head)] = rb
                        def f2(qc=qc, head=head):
                            nc.vector.tensor_mul(
                                ao[64 * head:64 * (head + 1), hp,
                                   qc * 512:(qc + 1) * 512],
                                ocs[(qc, head)][0:64, :], rbs[(qc, head)],
                            )
                        cls.append((f1, f2))
                    (a1, a2), (b1, b2) = cls[1], cls[2]
                    return [cls[0], a1, b1, a2, b2]

                eq = list(earlies)
                fq = list(fills)
                debt = 0.0
                prev = None
                for step in range(QC * KT):
                    qc, kt = divmod(step, KT)
                    ex = scores_exp(qc, kt)
                    if step == 9:
                        # qc0 of this unit is fully PV'd (step 8): normalize
                        # it in-unit, then any late fills (kept FIFO-after
                        # the norm closures they depend on)
                        fq.extend(Fill(c, 0.5) for c in norm_closures(0))
                        fq.extend(late_fills)
                    if eq:
                        eq.pop(0)()
                    steps_left = QC * KT - step
                    total_left = sum(fl.cost for fl in fq)
                    debt = min(debt + max(total_left / steps_left, min_rate),
                               14.0)
                    while fq and fq[0].cost <= debt:
                        fl = fq.pop(0)
                        debt -= fl.cost
                        fl.fn()
                    if prev is not None:
                        pv(*prev)
                    prev = (qc, kt, ex)
                pv(*prev)
                for f in eq:
                    f()
                for fl in fq:
                    fl.fn()
                return norm_closures(1)

            # warm the HAM clock gate during the DMA-bound head: ~3.4us
            # of dummy matmuls in the otherwise-idle 6.5-11us window so the
            # first real matmuls run at 2.4GHz instead of 1.2
            wrm = p_warm.tile([128, 512], BF16)
            nc.vector.memset(wrm[:], 1.0)
            for i in range(13):
                wp = p_gen.tile([128, 512], FP32, tag="gen",
                                name=f"warm{i}")
                nc.tensor.matmul(wp[:], wrm[:, 0:128], wrm[:, :],
                                 start=True, stop=True)

            # ---- schedule ----
            load_xt(0)
            alloc_v(0)
            qk00 = qkgen_fills(0, 0)
            for fl in qk00:
                fl.fn()

            # fills per unit: qkgen of the next unit everywhere, plus
            # vgen(next batch) on (b,2..5) and proj(prev batch) on (b,0..3).
            earlies = []
            norm_last = None
            for b in range(B):
                ao = p_ao.tile([128, DT, S], BF16, tag="ao")
                aos[b] = ao
                for hp in range(HP):
                    fills = []
                    late = []
                    min_rate = 0.0
                    if hp == 1 and b + 1 < B:
                        load_xt(b + 1)
                        alloc_v(b + 1)
                    if b == 0 and hp == 0:
                        fills += vgen_fills(0)
                        min_rate = 13.0
                    if not (b == B - 1 and hp == HP - 1):
                        nb, nhp = (b, hp + 1) if hp + 1 < HP else (b + 1, 0)
                        fills += qkgen_fills(nb, nhp)
                    if b + 1 < B and hp >= 1:
                        vg = vgen_fills(b + 1) if hp == 1 else vgs
                        vgs = vg
                        lo, hi = [(0, 3), (3, 6), (6, 9), (9, 12),
                                  (12, 16)][hp - 1]
                        fills += vg[lo:hi]
                    if b > 0 and hp <= 4:
                        tts = [(0, 1), (2, 3), (4, 5), (6,), (7,)][hp]
                        for t in tts:
                            fills += proj_fills_tt(b - 1, t)
                    if b == B - 1 and hp == HP - 1:
                        for tt in range(4):
                            late += proj_fills_tt(b, tt)
                        for tt in range(4, TT):
                            late += proj_fills_tt(b, tt, defer_j5=True)
                    norm_last = unit(b, hp, fills, earlies,
                                     min_rate=min_rate, late_fills=late)
                    earlies = norm_last
            for f in earlies:
                f()
            for tt in range(4, TT):
                proj_finish_tt(B - 1, tt)
    nc.finalize()
    return nc


def _marshal(x, W_qkv, W_out, b_out):
    bf = ml_dtypes.bfloat16
    wqkvT = np.ascontiguousarray(W_qkv.T).astype(bf)
    woutT = np.ascontiguousarray(W_out.T).astype(bf)
    bfull = np.ascontiguousarray(np.broadcast_to(
        np.asarray(b_out, np.float32).reshape(1, D), (128, D)))
    in_maps = []
    for c in range(N_CORES):
        xc = np.ascontiguousarray(
            np.asarray(x)[B * c:B * (c + 1)].reshape(B * S, D).T
        ).astype(bf)
        in_maps.append({
            "xT": xc, "wqkvT": wqkvT, "woutT": woutT, "biasf": bfull,
        })
    return in_maps


def run(x, W_qkv, W_out, b_out, trace=False, **spmd_kwargs):
    if "nc" not in _CACHE:
        _CACHE["nc"] = build_nc()
    nc = _CACHE["nc"]
    in_maps = _marshal(x, W_qkv, W_out, b_out)
    res = run_bass_kernel_spmd(
        nc, in_maps, core_ids=list(range(N_CORES)), trace=trace, **spmd_kwargs
    )
    out = np.stack([res.results[c]["y"] for c in range(N_CORES)], axis=0)
    out = out.reshape(N_CORES * B, S, D)
    return out, res


def kernel(x, W_qkv, W_out, b_out):
    out, _ = run(x, W_qkv, W_out, b_out)
    return out
